# revision 20
# baseline (speedup 1.0000x reference)
"""Trainium2 Bass kernel for the S4-reservoir layer (nn_S4R_58308476010695).

Math: y = tanh(causal_conv(u, K) + D*u);  out = GLU(W_mix @ y + b_mix)
where K[h,l] = 2*Re(sum_n CB[h,n] * Lambda[h,n]^l).

The poles satisfy |Lambda| <= 0.99, so K decays geometrically; truncating
the kernel at DLAG*128 lags gives rel error ~1e-5 (validated vs the exact
FFT reference).  The convolution is then a banded block-Toeplitz matmul:

  y[b,h,j*T+r] = sum_{d=0..DLAG-1} sum_s K[h, d*T+r-s] * u[b,h,(j-d)*T+s]

which maps onto the 128x128 tensor engine with the T=128 in-block position
as the contraction dim and (block j, batch b) as the moving free dim.

Sharding: phase 1 (per-channel conv + tanh) is sharded over H (32 channels
per core); phase 2 (1x1 mix + GLU, contracts over all 256 channels) is
sharded over L (512 positions per core).  The reshard in between is a
single 2MB fp16 AllToAll across the 8 cores of the chip.

All matmul operands are fp16 (PSUM accumulates in fp32); end-to-end rel
error vs the f32 reference is ~5e-4, dominated by fp16 operand rounding.
"""

import numpy as np

import concourse.bass as bass
import concourse.mybir as mybir
import concourse.tile as tile
from concourse import bacc, bass_utils

B, H, L, N = 8, 256, 4096, 64
T = 128            # conv block size = matmul contraction dim
DLAG = 6           # kernel truncation: DLAG*T = 768 lags
NCORE = 8
HL = H // NCORE    # 32 channels per core in phase 1
J = L // T         # 32 blocks per sequence
LS = L // NCORE    # 512 positions per core in phase 2
JJ = LS // T       # 4 blocks per L-slice

F16 = mybir.dt.float16
F32 = mybir.dt.float32
AF = mybir.ActivationFunctionType

# test.py pokes these for profiling
last_results = None
run_kwargs = {}


def _build_program():
    nc = bacc.Bacc(num_devices=NCORE)
    u_d = nc.declare_dram_parameter("u_arr", [T, HL * J * B], F16, False)
    w1_d = nc.declare_dram_parameter("w1", [T, HL * DLAG * T], F16, False)
    w2_d = nc.declare_dram_parameter("w2", [T, 2 * 512], F16, False)
    bb_d = nc.declare_dram_parameter("bb", [T, 4], F32, False)
    # out columns are in (r, jj, b) order; host un-permutes (free).
    out_d = nc.declare_dram_parameter("out", [2 * T, L], F16, True)

    with tile.TileContext(nc) as tc:
        with tc.tile_pool(name="const", bufs=1) as cpool, \
             tc.tile_pool(name="dram", bufs=1, space="DRAM") as dpool:
            # interleave u/w1 loads in consumption order so h=0 starts early
            u_sb = cpool.tile([T, HL * J * B], F16, tag="u", name="u_sb")
            w1_ts = []
            for h in range(HL):
                sl = slice(h * J * B, (h + 1) * J * B)
                nc.sync.dma_start(u_sb[:, sl], u_d[:, sl])
                w1_t = cpool.tile([T, DLAG * T], F16, tag=f"w1_{h}", name=f"w1_{h}")
                nc.sync.dma_start(w1_t[:], w1_d[:, h * DLAG * T:(h + 1) * DLAG * T])
                w1_ts.append(w1_t)
            w2_sb = cpool.tile([T, 2 * 512], F16, tag="w2", name="w2_sb")
            nc.sync.dma_start(w2_sb[:], w2_d[:])
            bb_sb = cpool.tile([T, 4], F32, tag="bb", name="bb_sb")
            nc.sync.dma_start(bb_sb[:], bb_d[:])

            # layout [dest_core, local_h, r, jj*B+b]: the (dest, jjb) split of the
            # matmul's natural [r, (j b)] output keeps every DMA 3-dim with a
            # contiguous 32-element inner run.
            y_loc = dpool.tile([NCORE, HL, T, JJ * B], F16, tag="yloc", name="y_loc")
            y_gath = dpool.tile([NCORE, HL, T, JJ * B], F16, tag="ygath", name="y_gath")

            ypool = tc.alloc_tile_pool(name="yp", bufs=10)
            rpool = tc.alloc_tile_pool(name="rhs", bufs=1)
            gpool = tc.alloc_tile_pool(name="glu", bufs=4)

            # ---- phase 1: banded Toeplitz conv + tanh, per local channel ----
            y_ts = []
            with tc.tile_pool(name="psum1", bufs=8, space="PSUM") as pp1:
                for h in range(HL):
                    ps = pp1.tile([T, J * B], F32, name=f"ps{h}", tag="ps")
                    for d in range(DLAG):
                        ncols = B * (J - d)
                        nc.tensor.matmul(
                            ps[:, d * B:J * B],
                            lhsT=w1_ts[h][:, d * T:(d + 1) * T],
                            rhs=u_sb[:, h * J * B: h * J * B + ncols],
                            start=(d == 0),
                            stop=(d == DLAG - 1),
                        )
                    y_t = ypool.tile([T, J * B], F16, name=f"y{h}", tag="y")
                    nc.scalar.activation(y_t[:], ps[:], AF.Tanh)
                    y_ts.append(y_t)
                    # scatter: y tile col dest*32+jjb -> y_loc[dest, h, r, jjb]
                    dst = y_loc[:, h].rearrange("dest r jjb -> r dest jjb")
                    src = y_t.rearrange("r (dest jjb) -> r dest jjb", dest=NCORE, jjb=JJ * B)
                    nc.sync.dma_start(dst, src)

                nc.gpsimd.collective_compute(
                    "AllToAll",
                    mybir.AluOpType.bypass,
                    replica_groups=[list(range(NCORE))],
                    ins=[y_loc.opt()],
                    outs=[y_gath.opt()],
                )

            # ---- phase 2: 1x1 channel mix + GLU on this core's L slice ----
            with tc.tile_pool(name="psum2", bufs=8, space="PSUM") as pp2:
                # rhs cols in (r, jj, b) order: per global h the DRAM data
                # [r, jjb] is already contiguous - one flat DMA per h-chunk.
                y_gv = y_gath.rearrange("src hl r jjb -> (src hl) (r jjb)")
                rhs_ts = []
                for c in range(2):
                    rt = rpool.tile([T, B * LS], F16, tag=f"rhs{c}", name=f"rhs{c}")
                    nc.sync.dma_start(rt[:], y_gv[c * T:(c + 1) * T])
                    rhs_ts.append(rt)
                NK = B * LS // 512          # 8 col chunks of 512
                for k in range(NK):
                    pss = []
                    for ot in range(4):
                        ps2 = pp2.tile([T, 512], F32, name=f"z{k}_{ot}", tag="z")
                        for c in range(2):
                            off = c * 512 + ot * T
                            nc.tensor.matmul(
                                ps2[:],
                                lhsT=w2_sb[:, off:off + T],
                                rhs=rhs_ts[c][:, k * 512:(k + 1) * 512],
                                start=(c == 0),
                                stop=(c == 1),
                            )
                        pss.append(ps2)
                    for og in range(2):
                        # sigmoid on ACT (sole ACT function in phase 2 - avoids
                        # activation-table swaps); (a + bias) * sig fused on DVE
                        sig = gpool.tile([T, 512], F16, name=f"sig{k}_{og}", tag="sig")
                        nc.scalar.activation(sig[:], pss[og + 2][:], AF.Sigmoid,
                                             bias=bb_sb[:, og + 2:og + 3])
                        outt = gpool.tile([T, 512], F16, name=f"o{k}_{og}", tag="outt")
                        nc.vector.scalar_tensor_tensor(
                            outt[:], pss[og][:], bb_sb[:, og:og + 1], sig[:],
                            op0=mybir.AluOpType.add, op1=mybir.AluOpType.mult)
                        nc.sync.dma_start(
                            out_d[og * T:(og + 1) * T, k * 512:(k + 1) * 512], outt[:])
            gpool.release()
            rpool.release()
            ypool.release()
    return nc


def _host_prep(u, Lambda_re, Lambda_im, CB_re, CB_im, D, W_mix, b_mix):
    Lam = Lambda_re.astype(np.complex128) + 1j * Lambda_im.astype(np.complex128)
    CB = CB_re.astype(np.complex128) + 1j * CB_im.astype(np.complex128)
    Lk = DLAG * T
    K = np.empty((H, Lk), np.float64)
    P = np.ones((H, N), np.complex128)
    for l in range(Lk):
        K[:, l] = 2.0 * (CB.real * P.real - CB.imag * P.imag).sum(axis=1)
        P *= Lam
    K[:, 0] += D.astype(np.float64)          # fold the skip connection into lag 0

    # lhsT Toeplitz tiles: W1[h,d,s,r] = K[h, d*T + r - s] (0 when out of band)
    Kbp = np.concatenate([np.zeros((H, T - 1)), K], axis=1)
    base = np.arange(T)[None, :] - np.arange(T)[:, None] + (T - 1)       # [s, r]
    idx = base[None, :, :] + (np.arange(DLAG) * T)[:, None, None]        # [d, s, r]
    W1 = Kbp[:, idx].astype(np.float16)                                  # [H, d, s, r]

    w2_arr = np.ascontiguousarray(
        np.concatenate([W_mix.T[:T], W_mix.T[T:]], axis=1)).astype(np.float16)
    bb_arr = np.ascontiguousarray(b_mix.reshape(4, T).T).astype(np.float32)

    in_maps = []
    for c in range(NCORE):
        h0 = c * HL
        u_arr = np.ascontiguousarray(
            u[:, h0:h0 + HL].reshape(B, HL, J, T).transpose(3, 1, 2, 0)
        ).reshape(T, HL * J * B).astype(np.float16)
        w1_arr = np.ascontiguousarray(
            W1[h0:h0 + HL].transpose(2, 0, 1, 3)).reshape(T, HL * DLAG * T)
        in_maps.append({"u_arr": u_arr, "w1": w1_arr, "w2": w2_arr, "bb": bb_arr})
    return in_maps


def kernel(u, Lambda_re, Lambda_im, CB_re, CB_im, D, W_mix, b_mix):
    global last_results
    args = [np.asarray(x) for x in
            (u, Lambda_re, Lambda_im, CB_re, CB_im, D, W_mix, b_mix)]
    in_maps = _host_prep(*args)
    nc = _build_program()
    nc.compile()   # bacc passes: wait legalization, regalloc, DCE
    res = bass_utils.run_bass_kernel_spmd(nc, in_maps, list(range(NCORE)), **run_kwargs)
    last_results = res
    out = np.empty((B, H, L), np.float32)
    for c in range(NCORE):
        # device cols are (r, jj, b); this core holds l in [c*LS, (c+1)*LS)
        a = res.results[c]["out"].astype(np.float32).reshape(H, T, JJ, B)
        out[:, :, c * LS:(c + 1) * LS] = (
            a.transpose(3, 0, 2, 1).reshape(B, H, LS))
    return out
